# revision 10
# baseline (speedup 1.0000x reference)
"""LIFNet (SNN: fc1 -> LIF -> fc2 -> LIF -> rate readout) on 8 TRN2 cores.

Sharding: tensor-parallel over hidden dim (4096 -> 512/core). Each core:
  fc1 shard:  c = x @ W1[:, h] + b1[h]           ([512B, 3072D] @ [3072, 512])
  LIF1 scan over T=16 (input constant across t)  -> spikes s_t [512H, 512B]
  fc2 partial: y_t = W2[h]^T @ s_t               ([10, 512B] per t, pre-bias)
Host: all-reduce y partials (sum), + b2, LIF2 + rate readout + loss (tiny).

Device layout is transposed: activations live as [H_part, B_free] so fc1
output feeds LIF elementwise ops and fc2 matmuls without any transposes.
"""

import sys

sys.path.insert(0, "/opt/trn_rl_repo")

import numpy as np

import concourse.bass as bass
import concourse.bacc as bacc
import concourse.mybir as mybir
from concourse.tile import TileContext
from concourse.bass_utils import run_bass_kernel_spmd

P = 128          # SBUF partitions
B = 512          # full batch (replicated on every core)
D = 3072         # input features
H = 4096         # hidden
O = 10           # output classes
T = 16           # LIF time steps
NCORES = 8
HS = H // NCORES         # 512 hidden per core
KD = D // P              # 24 contraction tiles for fc1
MH = HS // P             # 4 hidden tiles per core
TG = 4                   # time-step group size (PSUM: 4 c-banks + 4 y-banks)
NG = T // TG
F32 = mybir.dt.float32
BF16 = mybir.dt.bfloat16

_cache = {}
last_results = None      # BassKernelResults of the most recent run (for test.py)


def _build_nc():
    nc = bacc.Bacc(target_bir_lowering=False)
    xh = nc.declare_dram_parameter("xh", [D, B], BF16, isOutput=False)
    xl = nc.declare_dram_parameter("xl", [D, B], BF16, isOutput=False)
    w1h = nc.declare_dram_parameter("w1h", [D, HS], BF16, isOutput=False)
    w1l = nc.declare_dram_parameter("w1l", [D, HS], BF16, isOutput=False)
    b1c = nc.declare_dram_parameter("b1c", [P, MH], F32, isOutput=False)
    w2h = nc.declare_dram_parameter("w2h", [HS, O], BF16, isOutput=False)
    w2l = nc.declare_dram_parameter("w2l", [HS, O], BF16, isOutput=False)
    y = nc.declare_dram_parameter("y", [O, T * B], F32, isOutput=True)

    with TileContext(nc) as tc:
        with (
            tc.tile_pool(name="xpool", bufs=1) as xpool,
            tc.tile_pool(name="wpool", bufs=3) as wpool,
            tc.tile_pool(name="spool", bufs=2) as spool,
            tc.tile_pool(name="uqpool", bufs=1) as uqpool,
            tc.tile_pool(name="cpool", bufs=1) as cpool,
            tc.tile_pool(name="misc", bufs=1) as misc,
            tc.tile_pool(name="psum_c", bufs=1, space="PSUM") as pc,
            tc.tile_pool(name="psum_y", bufs=1, space="PSUM") as py,
        ):
            # resident inputs. x loads as ONE DMA per half (one sem each).
            xsbh = xpool.tile([P, KD, B], BF16, tag="xsbh", name="xsbh")
            nc.sync.dma_start(
                xsbh[:, :, :],
                xh.rearrange("(k p) b -> p k b", p=P)[:, :, :])
            xsbl = xpool.tile([P, KD, B], BF16, tag="xsbl", name="xsbl")
            nc.sync.dma_start(
                xsbl[:, :, :],
                xl.rearrange("(k p) b -> p k b", p=P)[:, :, :])
            w2t = []
            for m in range(MH):
                w2hm = misc.tile([P, O], BF16, tag=f"w2h{m}", name=f"w2h{m}")
                nc.sync.dma_start(w2hm[:, :], w2h[m * P:(m + 1) * P, :])
                w2lm = misc.tile([P, O], BF16, tag=f"w2l{m}", name=f"w2l{m}")
                nc.sync.dma_start(w2lm[:, :], w2l[m * P:(m + 1) * P, :])
                w2t.append((w2hm, w2lm))
            b1sb = misc.tile([P, MH], F32, tag="b1", name="b1sb")
            nc.sync.dma_start(b1sb[:, :], b1c[:, :])
            scr = misc.tile([1, 4], F32, tag="scr", name="scr")
            # DVE pre-touch of b1 so later DVE ops need only the PE wait
            nc.vector.tensor_copy(scr[0:1, 0:1], b1sb[0:1, 0:1])

            # PE pre-touch dummies: absorb each input-DMA wait into its own
            # 1-wait matmul (fp32 Matmult supports at most one sync wait).
            dmy = py.tile([O, B], F32, tag="y0", name="dmy")
            for pre in (xsbh[:, 0, 0:1], xsbl[:, 0, 0:1]):
                nc.tensor.matmul(dmy[0:1, 0:1], pre, pre,
                                 start=True, stop=True, skip_group_check=True)
            for m in range(MH):
                for pre in (w2t[m][0][:, 0:1], w2t[m][1][:, 0:1]):
                    nc.tensor.matmul(dmy[0:1, 0:1], pre, pre,
                                     start=True, stop=True, skip_group_check=True)

            # fc1: accumulate c^T[m] = sum_k W1[k,m]^T @ xT[k] in PSUM
            cps = [pc.tile([P, B], F32, tag=f"c{m}", name=f"c{m}") for m in range(MH)]
            for k in range(KD):
                w1kh = wpool.tile([P, HS], BF16, tag="w1kh", name=f"w1kh{k}")
                nc.sync.dma_start(w1kh[:, :], w1h[k * P:(k + 1) * P, :])
                w1kl = wpool.tile([P, HS], BF16, tag="w1kl", name=f"w1kl{k}")
                nc.sync.dma_start(w1kl[:, :], w1l[k * P:(k + 1) * P, :])
                for m in range(MH):
                    sl = slice(m * P, (m + 1) * P)
                    terms = [(w1kh, xsbh), (w1kh, xsbl), (w1kl, xsbh)]
                    for i, (wt, xt) in enumerate(terms):
                        nc.tensor.matmul(
                            cps[m][:, :], wt[:, sl], xt[:, k, :],
                            start=(k == 0 and i == 0),
                            stop=(k == KD - 1 and i == len(terms) - 1),
                        )

            # c' = 0.5*(c + b1), all MH tiles packed wide so scan ops are
            # single [P, MH*B] DVE instructions (amortizes per-op overhead)
            cpt = cpool.tile([P, MH * B], F32, tag="cp", name="cpt")
            for m in range(MH):
                nc.vector.tensor_scalar(
                    cpt[:, m * B:(m + 1) * B], cps[m][:, :],
                    b1sb[:, m:m + 1], 0.5,
                    mybir.AluOpType.add, mybir.AluOpType.mult,
                )

            # membrane state (scaled: holds 0.5 * v_after_reset)
            mst = cpool.tile([P, MH * B], F32, tag="mst", name="mst")
            nc.vector.memset(mst[:, :], 0.0)

            ysb = misc.tile([O, T * B], F32, tag="ysb", name="ysb")

            for g in range(NG):
                stiles = {}
                for ti in range(TG):
                    u = uqpool.tile([P, MH * B], F32, tag="u", name=f"u{g}_{ti}")
                    s = spool.tile([P, MH * B], BF16, tag=f"s{ti}", name=f"s{g}_{ti}")
                    q = uqpool.tile([P, MH * B], F32, tag="q", name=f"q{g}_{ti}")
                    # u = v_t (pre-reset) ; s = spike ; mst = 0.5*v*(no spike)
                    nc.vector.tensor_add(u[:, :], mst[:, :], cpt[:, :])
                    nc.vector.tensor_scalar(
                        s[:, :], u[:, :], 1.0, None, mybir.AluOpType.is_ge)
                    nc.vector.tensor_scalar(
                        q[:, :], u[:, :], 1.0, 0.5,
                        mybir.AluOpType.is_lt, mybir.AluOpType.mult)
                    nc.vector.tensor_mul(mst[:, :], u[:, :], q[:, :])
                    stiles[ti] = s
                for ti in range(TG):
                    t = g * TG + ti
                    yp = py.tile([O, B], F32, tag=f"y{ti}", name=f"yp{g}_{ti}")
                    for m in range(MH):
                        for i in range(2):
                            nc.tensor.matmul(
                                yp[:, :], w2t[m][i][:, :],
                                stiles[ti][:, m * B:(m + 1) * B],
                                start=(m == 0 and i == 0),
                                stop=(m == MH - 1 and i == 1),
                            )
                    nc.vector.tensor_copy(ysb[:, t * B:(t + 1) * B], yp[:, :])

            nc.sync.dma_start(y[:, :], ysb[:, :])
    if not nc.is_finalized():
        nc.finalize()
    return nc


def _get_nc():
    if "nc" not in _cache:
        _cache["nc"] = _build_nc()
    return _cache["nc"]


def kernel(input, labels, W1, b1, W2, b2, trace=False):
    global last_results
    import ml_dtypes
    bf = ml_dtypes.bfloat16

    def split(a):
        hi = a.astype(bf)
        lo = (a.astype(np.float32) - hi.astype(np.float32)).astype(bf)
        return hi, lo

    x = input.reshape(B, D).astype(np.float32)
    xT = np.ascontiguousarray(x.T)
    xTh, xTl = split(xT)
    xTh, xTl = np.ascontiguousarray(xTh), np.ascontiguousarray(xTl)
    W1h, W1l = split(W1.astype(np.float32))
    W2h, W2l = split(W2.astype(np.float32))
    in_maps = []
    for c in range(NCORES):
        h0 = c * HS
        hsl = slice(h0, h0 + HS)
        b1s = np.ascontiguousarray(
            (0.5 * b1[hsl]).astype(np.float32).reshape(MH, P).T)
        in_maps.append({
            "xh": xTh, "xl": xTl,
            "w1h": np.ascontiguousarray(W1h[:, hsl]),
            "w1l": np.ascontiguousarray(W1l[:, hsl]),
            "b1c": b1s,
            "w2h": np.ascontiguousarray(W2h[hsl, :]),
            "w2l": np.ascontiguousarray(W2l[hsl, :]),
        })

    nc = _get_nc()
    last_results = run_bass_kernel_spmd(
        nc, in_maps, list(range(NCORES)), trace=trace)
    ys = np.stack([r["y"].reshape(O, T, B) for r in last_results.results], 0)
    ysum = ys.sum(axis=0, dtype=np.float32)          # [O, T, B]
    yt = ysum.transpose(1, 2, 0) + b2.astype(np.float32)   # [T, B, O]

    # LIF2 + rate readout + loss (tiny; mirrors reference fp32 op order)
    v = np.zeros((B, O), np.float32)
    ssum = np.zeros((B, O), np.float32)
    half = np.float32(2.0)
    for t in range(T):
        v = v + (yt[t] - v) / half
        s = (v - np.float32(1.0) >= 0).astype(np.float32)
        v = v * (np.float32(1.0) - s)
        ssum = ssum + s
    pred = ssum / np.float32(T)
    diff = pred - labels.astype(np.float32)
    loss = np.float32(np.mean(diff * diff, dtype=np.float32))
    return pred, loss


# revision 11
# speedup vs baseline: 1.1237x; 1.1237x over previous
"""LIFNet (SNN: fc1 -> LIF -> fc2 -> LIF -> rate readout) on 8 TRN2 cores.

Sharding: tensor-parallel over hidden dim (4096 -> 512/core). Each core:
  fc1 shard:  c = x @ W1[:, h] + b1[h]           ([512B, 3072D] @ [3072, 512])
  LIF1 scan over T=16 (input constant across t)  -> spikes s_t [512H, 512B]
  fc2 partial: y_t = W2[h]^T @ s_t               ([10, 512B] per t, pre-bias)
Host: all-reduce y partials (sum), + b2, LIF2 + rate readout + loss (tiny).

Device layout is transposed: activations live as [H_part, B_free] so fc1
output feeds LIF elementwise ops and fc2 matmuls without any transposes.
"""

import sys

sys.path.insert(0, "/opt/trn_rl_repo")

import numpy as np

import concourse.bass as bass
import concourse.bacc as bacc
import concourse.mybir as mybir
from concourse.tile import TileContext
from concourse.bass_utils import run_bass_kernel_spmd

P = 128          # SBUF partitions
B = 512          # full batch (replicated on every core)
D = 3072         # input features
H = 4096         # hidden
O = 10           # output classes
T = 16           # LIF time steps
NCORES = 8
HS = H // NCORES         # 512 hidden per core
KD = D // P              # 24 contraction tiles for fc1
MH = HS // P             # 4 hidden tiles per core
TG = 4                   # time-step group size (PSUM: 4 c-banks + 4 y-banks)
NG = T // TG
F32 = mybir.dt.float32
BF16 = mybir.dt.bfloat16

_cache = {}
last_results = None      # BassKernelResults of the most recent run (for test.py)


def _build_nc():
    nc = bacc.Bacc(target_bir_lowering=False)
    xh = nc.declare_dram_parameter("xh", [D, B], BF16, isOutput=False)
    xl = nc.declare_dram_parameter("xl", [D, B], BF16, isOutput=False)
    w1h = nc.declare_dram_parameter("w1h", [D, HS], BF16, isOutput=False)
    w1l = nc.declare_dram_parameter("w1l", [D, HS], BF16, isOutput=False)
    b1c = nc.declare_dram_parameter("b1c", [P, MH], F32, isOutput=False)
    w2h = nc.declare_dram_parameter("w2h", [HS, O], BF16, isOutput=False)
    w2l = nc.declare_dram_parameter("w2l", [HS, O], BF16, isOutput=False)
    y = nc.declare_dram_parameter("y", [O, T * B], F32, isOutput=True)

    with TileContext(nc) as tc:
        with (
            tc.tile_pool(name="xpool", bufs=1) as xpool,
            tc.tile_pool(name="wpool", bufs=3) as wpool,
            tc.tile_pool(name="spool", bufs=2) as spool,
            tc.tile_pool(name="uqpool", bufs=1) as uqpool,
            tc.tile_pool(name="cpool", bufs=1) as cpool,
            tc.tile_pool(name="misc", bufs=1) as misc,
            tc.tile_pool(name="psum_c", bufs=1, space="PSUM") as pc,
            tc.tile_pool(name="psum_y", bufs=1, space="PSUM") as py,
        ):
            # resident inputs. x loads as ONE DMA per half (one sem each).
            xsbh = xpool.tile([P, KD, B], BF16, tag="xsbh", name="xsbh")
            nc.sync.dma_start(
                xsbh[:, :, :],
                xh.rearrange("(k p) b -> p k b", p=P)[:, :, :])
            xsbl = xpool.tile([P, KD, B], BF16, tag="xsbl", name="xsbl")
            nc.sync.dma_start(
                xsbl[:, :, :],
                xl.rearrange("(k p) b -> p k b", p=P)[:, :, :])
            w2t = []
            for m in range(MH):
                w2hm = misc.tile([P, O], BF16, tag=f"w2h{m}", name=f"w2h{m}")
                nc.sync.dma_start(w2hm[:, :], w2h[m * P:(m + 1) * P, :])
                w2lm = misc.tile([P, O], BF16, tag=f"w2l{m}", name=f"w2l{m}")
                nc.sync.dma_start(w2lm[:, :], w2l[m * P:(m + 1) * P, :])
                w2t.append((w2hm, w2lm))
            b1sb = misc.tile([P, MH], F32, tag="b1", name="b1sb")
            nc.sync.dma_start(b1sb[:, :], b1c[:, :])
            scr = misc.tile([1, 4], F32, tag="scr", name="scr")
            # DVE pre-touch of b1 so later DVE ops need only the PE wait
            nc.vector.tensor_copy(scr[0:1, 0:1], b1sb[0:1, 0:1])

            # PE pre-touch dummies: absorb each input-DMA wait into its own
            # 1-wait matmul (fp32 Matmult supports at most one sync wait).
            dmy = py.tile([O, B], F32, tag="y0", name="dmy")
            for pre in (xsbh[:, 0, 0:1], xsbl[:, 0, 0:1]):
                nc.tensor.matmul(dmy[0:1, 0:1], pre, pre,
                                 start=True, stop=True, skip_group_check=True)
            for m in range(MH):
                for pre in (w2t[m][0][:, 0:1], w2t[m][1][:, 0:1]):
                    nc.tensor.matmul(dmy[0:1, 0:1], pre, pre,
                                     start=True, stop=True, skip_group_check=True)

            # fc1: accumulate c^T[m] = sum_k W1[k,m]^T @ xT[k] in PSUM
            cps = [pc.tile([P, B], F32, tag=f"c{m}", name=f"c{m}") for m in range(MH)]

            def fc1_half(h):
                HH = HS // 2
                c0 = h * HH
                for k in range(KD):
                    w1kh = wpool.tile([P, HH], BF16, tag=f"w1kh{h}",
                                      name=f"w1kh{h}_{k}")
                    nc.sync.dma_start(
                        w1kh[:, :], w1h[k * P:(k + 1) * P, c0:c0 + HH])
                    w1kl = wpool.tile([P, HH], BF16, tag=f"w1kl{h}",
                                      name=f"w1kl{h}_{k}")
                    nc.sync.dma_start(
                        w1kl[:, :], w1l[k * P:(k + 1) * P, c0:c0 + HH])
                    for mi in range(MH // 2):
                        m = h * (MH // 2) + mi
                        sl = slice(mi * P, (mi + 1) * P)
                        terms = [(w1kh, xsbh), (w1kh, xsbl), (w1kl, xsbh)]
                        for i, (wt, xt) in enumerate(terms):
                            nc.tensor.matmul(
                                cps[m][:, :], wt[:, sl], xt[:, k, :],
                                start=(k == 0 and i == 0),
                                stop=(k == KD - 1 and i == len(terms) - 1),
                            )

            cpt = cpool.tile([P, MH * B], F32, tag="cp", name="cpt")
            mst = cpool.tile([P, MH * B], F32, tag="mst", name="mst")
            nc.vector.memset(mst[:, :], 0.0)
            ysb = misc.tile([O, T * B], F32, tag="ysb", name="ysb")
            HB = (MH // 2) * B     # half width in batch-packed columns

            stiles = {}
            for t in range(T):
                stiles[t] = spool.tile([P, MH * B], BF16, tag=f"s{t}",
                                       name=f"s{t}", bufs=1)

            def scan_half(h):
                # c' = 0.5*(c + b1) for this half's m tiles, then the full
                # 16-step LIF scan on [P, HB]-wide slices.
                hsl = slice(h * HB, (h + 1) * HB)
                for mi in range(MH // 2):
                    m = h * (MH // 2) + mi
                    nc.vector.tensor_scalar(
                        cpt[:, m * B:(m + 1) * B], cps[m][:, :],
                        b1sb[:, m:m + 1], 0.5,
                        mybir.AluOpType.add, mybir.AluOpType.mult,
                    )
                for t in range(T):
                    u = uqpool.tile([P, HB], F32, tag=f"u{h}", name=f"u{h}_{t}")
                    q = uqpool.tile([P, HB], F32, tag=f"q{h}", name=f"q{h}_{t}")
                    nc.vector.tensor_add(u[:, :], mst[:, hsl], cpt[:, hsl])
                    nc.vector.tensor_scalar(
                        stiles[t][:, hsl], u[:, :], 1.0, None,
                        mybir.AluOpType.is_ge)
                    nc.vector.tensor_scalar(
                        q[:, :], u[:, :], 1.0, 0.5,
                        mybir.AluOpType.is_lt, mybir.AluOpType.mult)
                    nc.vector.tensor_mul(mst[:, hsl], u[:, :], q[:, :])

            # half 0 fc1 -> half 0 scan overlaps half 1 fc1 on PE
            fc1_half(0)
            scan_half(0)
            fc1_half(1)
            scan_half(1)

            for g in range(NG):
                for ti in range(TG):
                    t = g * TG + ti
                    yp = py.tile([O, B], F32, tag=f"y{ti}", name=f"yp{g}_{ti}")
                    for m in range(MH):
                        for i in range(2):
                            nc.tensor.matmul(
                                yp[:, :], w2t[m][i][:, :],
                                stiles[t][:, m * B:(m + 1) * B],
                                start=(m == 0 and i == 0),
                                stop=(m == MH - 1 and i == 1),
                            )
                    nc.vector.tensor_copy(ysb[:, t * B:(t + 1) * B], yp[:, :])

            nc.sync.dma_start(y[:, :], ysb[:, :])
    if not nc.is_finalized():
        nc.finalize()
    return nc


def _get_nc():
    if "nc" not in _cache:
        _cache["nc"] = _build_nc()
    return _cache["nc"]


def kernel(input, labels, W1, b1, W2, b2, trace=False):
    global last_results
    import ml_dtypes
    bf = ml_dtypes.bfloat16

    def split(a):
        hi = a.astype(bf)
        lo = (a.astype(np.float32) - hi.astype(np.float32)).astype(bf)
        return hi, lo

    x = input.reshape(B, D).astype(np.float32)
    xT = np.ascontiguousarray(x.T)
    xTh, xTl = split(xT)
    xTh, xTl = np.ascontiguousarray(xTh), np.ascontiguousarray(xTl)
    W1h, W1l = split(W1.astype(np.float32))
    W2h, W2l = split(W2.astype(np.float32))
    in_maps = []
    for c in range(NCORES):
        h0 = c * HS
        hsl = slice(h0, h0 + HS)
        b1s = np.ascontiguousarray(
            (0.5 * b1[hsl]).astype(np.float32).reshape(MH, P).T)
        in_maps.append({
            "xh": xTh, "xl": xTl,
            "w1h": np.ascontiguousarray(W1h[:, hsl]),
            "w1l": np.ascontiguousarray(W1l[:, hsl]),
            "b1c": b1s,
            "w2h": np.ascontiguousarray(W2h[hsl, :]),
            "w2l": np.ascontiguousarray(W2l[hsl, :]),
        })

    nc = _get_nc()
    last_results = run_bass_kernel_spmd(
        nc, in_maps, list(range(NCORES)), trace=trace)
    ys = np.stack([r["y"].reshape(O, T, B) for r in last_results.results], 0)
    ysum = ys.sum(axis=0, dtype=np.float32)          # [O, T, B]
    yt = ysum.transpose(1, 2, 0) + b2.astype(np.float32)   # [T, B, O]

    # LIF2 + rate readout + loss (tiny; mirrors reference fp32 op order)
    v = np.zeros((B, O), np.float32)
    ssum = np.zeros((B, O), np.float32)
    half = np.float32(2.0)
    for t in range(T):
        v = v + (yt[t] - v) / half
        s = (v - np.float32(1.0) >= 0).astype(np.float32)
        v = v * (np.float32(1.0) - s)
        ssum = ssum + s
    pred = ssum / np.float32(T)
    diff = pred - labels.astype(np.float32)
    loss = np.float32(np.mean(diff * diff, dtype=np.float32))
    return pred, loss


# revision 12
# speedup vs baseline: 1.2023x; 1.0700x over previous
"""LIFNet (SNN: fc1 -> LIF -> fc2 -> LIF -> rate readout) on 8 TRN2 cores.

Sharding: tensor-parallel over hidden dim (4096 -> 512/core). Each core:
  fc1 shard:  c = x @ W1[:, h] + b1[h]           ([512B, 3072D] @ [3072, 512])
  LIF1 scan over T=16 (input constant across t)  -> spikes s_t [512H, 512B]
  fc2 partial: y_t = W2[h]^T @ s_t               ([10, 512B] per t, pre-bias)
Host: all-reduce y partials (sum), + b2, LIF2 + rate readout + loss (tiny).

Device layout is transposed: activations live as [H_part, B_free] so fc1
output feeds LIF elementwise ops and fc2 matmuls without any transposes.
"""

import sys

sys.path.insert(0, "/opt/trn_rl_repo")

import numpy as np

import concourse.bass as bass
import concourse.bacc as bacc
import concourse.mybir as mybir
from concourse.tile import TileContext
from concourse.bass_utils import run_bass_kernel_spmd

P = 128          # SBUF partitions
B = 512          # full batch (replicated on every core)
D = 3072         # input features
H = 4096         # hidden
O = 10           # output classes
T = 16           # LIF time steps
NCORES = 8
HS = H // NCORES         # 512 hidden per core
KD = D // P              # 24 contraction tiles for fc1
MH = HS // P             # 4 hidden tiles per core
TG = 4                   # time-step group size (PSUM: 4 c-banks + 4 y-banks)
NG = T // TG
F32 = mybir.dt.float32
BF16 = mybir.dt.bfloat16

_cache = {}
last_results = None      # BassKernelResults of the most recent run (for test.py)


def _build_nc():
    nc = bacc.Bacc(target_bir_lowering=False)
    xh = nc.declare_dram_parameter("xh", [D, B], BF16, isOutput=False)
    xl = nc.declare_dram_parameter("xl", [D, B], BF16, isOutput=False)
    w1h = nc.declare_dram_parameter("w1h", [D, HS], BF16, isOutput=False)
    w1l = nc.declare_dram_parameter("w1l", [D, HS], BF16, isOutput=False)
    b1c = nc.declare_dram_parameter("b1c", [P, MH], F32, isOutput=False)
    w2h = nc.declare_dram_parameter("w2h", [HS, O], BF16, isOutput=False)
    w2l = nc.declare_dram_parameter("w2l", [HS, O], BF16, isOutput=False)
    y = nc.declare_dram_parameter("y", [O, T * B], F32, isOutput=True)

    with TileContext(nc) as tc:
        with (
            tc.tile_pool(name="xpool", bufs=1) as xpool,
            tc.tile_pool(name="wpool", bufs=3) as wpool,
            tc.tile_pool(name="spool", bufs=2) as spool,
            tc.tile_pool(name="uqpool", bufs=1) as uqpool,
            tc.tile_pool(name="cpool", bufs=1) as cpool,
            tc.tile_pool(name="misc", bufs=1) as misc,
            tc.tile_pool(name="psum_c", bufs=1, space="PSUM") as pc,
            tc.tile_pool(name="psum_y", bufs=1, space="PSUM") as py,
        ):
            # resident inputs. x loads as ONE DMA per half (one sem each).
            xsbh = xpool.tile([P, KD, B], BF16, tag="xsbh", name="xsbh")
            nc.sync.dma_start(
                xsbh[:, :, :],
                xh.rearrange("(k p) b -> p k b", p=P)[:, :, :])
            xsbl = xpool.tile([P, KD, B], BF16, tag="xsbl", name="xsbl")
            nc.sync.dma_start(
                xsbl[:, :, :],
                xl.rearrange("(k p) b -> p k b", p=P)[:, :, :])
            w2t = []
            for m in range(MH):
                w2hm = misc.tile([P, O], BF16, tag=f"w2h{m}", name=f"w2h{m}")
                nc.sync.dma_start(w2hm[:, :], w2h[m * P:(m + 1) * P, :])
                w2lm = misc.tile([P, O], BF16, tag=f"w2l{m}", name=f"w2l{m}")
                nc.sync.dma_start(w2lm[:, :], w2l[m * P:(m + 1) * P, :])
                w2t.append((w2hm, w2lm))
            b1sb = misc.tile([P, MH], F32, tag="b1", name="b1sb")
            nc.sync.dma_start(b1sb[:, :], b1c[:, :])
            scr = misc.tile([1, 4], F32, tag="scr", name="scr")
            # DVE pre-touch of b1 so later DVE ops need only the PE wait
            nc.vector.tensor_copy(scr[0:1, 0:1], b1sb[0:1, 0:1])

            # PE pre-touch dummies: absorb each input-DMA wait into its own
            # 1-wait matmul (fp32 Matmult supports at most one sync wait).
            dmy = py.tile([O, B], F32, tag="y0", name="dmy")
            for pre in (xsbh[:, 0, 0:1], xsbl[:, 0, 0:1]):
                nc.tensor.matmul(dmy[0:1, 0:1], pre, pre,
                                 start=True, stop=True, skip_group_check=True)
            for m in range(MH):
                for pre in (w2t[m][0][:, 0:1], w2t[m][1][:, 0:1]):
                    nc.tensor.matmul(dmy[0:1, 0:1], pre, pre,
                                     start=True, stop=True, skip_group_check=True)

            # fc1: accumulate c^T[m] = sum_k W1[k,m]^T @ xT[k] in PSUM
            cps = [pc.tile([P, B], F32, tag=f"c{m}", name=f"c{m}") for m in range(MH)]

            def fc1_half(h):
                HH = HS // 2
                c0 = h * HH
                for k in range(KD):
                    w1kh = wpool.tile([P, HH], BF16, tag=f"w1kh{h}",
                                      name=f"w1kh{h}_{k}")
                    nc.sync.dma_start(
                        w1kh[:, :], w1h[k * P:(k + 1) * P, c0:c0 + HH])
                    w1kl = wpool.tile([P, HH], BF16, tag=f"w1kl{h}",
                                      name=f"w1kl{h}_{k}")
                    nc.sync.dma_start(
                        w1kl[:, :], w1l[k * P:(k + 1) * P, c0:c0 + HH])
                    for mi in range(MH // 2):
                        m = h * (MH // 2) + mi
                        sl = slice(mi * P, (mi + 1) * P)
                        terms = [(w1kh, xsbh), (w1kh, xsbl), (w1kl, xsbh)]
                        for i, (wt, xt) in enumerate(terms):
                            nc.tensor.matmul(
                                cps[m][:, :], wt[:, sl], xt[:, k, :],
                                start=(k == 0 and i == 0),
                                stop=(k == KD - 1 and i == len(terms) - 1),
                            )

            cpt = cpool.tile([P, MH * B], F32, tag="cp", name="cpt")
            mst = cpool.tile([P, MH * B], F32, tag="mst", name="mst")
            nc.vector.memset(mst[:, :], 0.0)
            ysb = misc.tile([O, T * B], F32, tag="ysb", name="ysb")
            HB = (MH // 2) * B     # half width in batch-packed columns

            stiles = {}
            for t in range(T):
                stiles[t] = spool.tile([P, MH * B], BF16, tag=f"s{t}",
                                       name=f"s{t}", bufs=1)

            def scan_half(h):
                # c' = 0.5*(c + b1) for this half's m tiles, then the full
                # 16-step LIF scan on [P, HB]-wide slices.
                hsl = slice(h * HB, (h + 1) * HB)
                for mi in range(MH // 2):
                    m = h * (MH // 2) + mi
                    nc.vector.tensor_scalar(
                        cpt[:, m * B:(m + 1) * B], cps[m][:, :],
                        b1sb[:, m:m + 1], 0.5,
                        mybir.AluOpType.add, mybir.AluOpType.mult,
                    )
                for t in range(T):
                    u = uqpool.tile([P, HB], F32, tag=f"u{h}", name=f"u{h}_{t}")
                    q = uqpool.tile([P, HB], F32, tag=f"q{h}", name=f"q{h}_{t}")
                    nc.vector.tensor_add(u[:, :], mst[:, hsl], cpt[:, hsl])
                    nc.vector.tensor_scalar(
                        stiles[t][:, hsl], u[:, :], 1.0, None,
                        mybir.AluOpType.is_ge)
                    nc.gpsimd.tensor_scalar(
                        q[:, :], u[:, :], 1.0, 0.5,
                        mybir.AluOpType.is_lt, mybir.AluOpType.mult)
                    nc.vector.tensor_mul(mst[:, hsl], u[:, :], q[:, :])

            # half 0 fc1 -> half 0 scan overlaps half 1 fc1 on PE
            fc1_half(0)
            scan_half(0)
            fc1_half(1)
            scan_half(1)

            for g in range(NG):
                for ti in range(TG):
                    t = g * TG + ti
                    yp = py.tile([O, B], F32, tag=f"y{ti}", name=f"yp{g}_{ti}")
                    for m in range(MH):
                        for i in range(2):
                            nc.tensor.matmul(
                                yp[:, :], w2t[m][i][:, :],
                                stiles[t][:, m * B:(m + 1) * B],
                                start=(m == 0 and i == 0),
                                stop=(m == MH - 1 and i == 1),
                            )
                    nc.vector.tensor_copy(ysb[:, t * B:(t + 1) * B], yp[:, :])

            nc.sync.dma_start(y[:, :], ysb[:, :])
    if not nc.is_finalized():
        nc.finalize()
    return nc


def _get_nc():
    if "nc" not in _cache:
        _cache["nc"] = _build_nc()
    return _cache["nc"]


def kernel(input, labels, W1, b1, W2, b2, trace=False):
    global last_results
    import ml_dtypes
    bf = ml_dtypes.bfloat16

    def split(a):
        hi = a.astype(bf)
        lo = (a.astype(np.float32) - hi.astype(np.float32)).astype(bf)
        return hi, lo

    x = input.reshape(B, D).astype(np.float32)
    xT = np.ascontiguousarray(x.T)
    xTh, xTl = split(xT)
    xTh, xTl = np.ascontiguousarray(xTh), np.ascontiguousarray(xTl)
    W1h, W1l = split(W1.astype(np.float32))
    W2h, W2l = split(W2.astype(np.float32))
    in_maps = []
    for c in range(NCORES):
        h0 = c * HS
        hsl = slice(h0, h0 + HS)
        b1s = np.ascontiguousarray(
            (0.5 * b1[hsl]).astype(np.float32).reshape(MH, P).T)
        in_maps.append({
            "xh": xTh, "xl": xTl,
            "w1h": np.ascontiguousarray(W1h[:, hsl]),
            "w1l": np.ascontiguousarray(W1l[:, hsl]),
            "b1c": b1s,
            "w2h": np.ascontiguousarray(W2h[hsl, :]),
            "w2l": np.ascontiguousarray(W2l[hsl, :]),
        })

    nc = _get_nc()
    last_results = run_bass_kernel_spmd(
        nc, in_maps, list(range(NCORES)), trace=trace)
    ys = np.stack([r["y"].reshape(O, T, B) for r in last_results.results], 0)
    ysum = ys.sum(axis=0, dtype=np.float32)          # [O, T, B]
    yt = ysum.transpose(1, 2, 0) + b2.astype(np.float32)   # [T, B, O]

    # LIF2 + rate readout + loss (tiny; mirrors reference fp32 op order)
    v = np.zeros((B, O), np.float32)
    ssum = np.zeros((B, O), np.float32)
    half = np.float32(2.0)
    for t in range(T):
        v = v + (yt[t] - v) / half
        s = (v - np.float32(1.0) >= 0).astype(np.float32)
        v = v * (np.float32(1.0) - s)
        ssum = ssum + s
    pred = ssum / np.float32(T)
    diff = pred - labels.astype(np.float32)
    loss = np.float32(np.mean(diff * diff, dtype=np.float32))
    return pred, loss
